# revision 26
# baseline (speedup 1.0000x reference)
"""
Trainium2 Bass kernel for nn_EquivariantProductBasisBlock (v4).

Math (per node n, channel c), species e = argmax(attrs[n]):
    s = feats[n,c,0]; v = feats[n,c,1:4]; s2 = s^2; v2 = |v|^2
    out0 = s*w00 + s2*w01 + v2*w02 + s^3*w03 + s*v2*w04   (w0p = w_paths0[e,p,c])
    c1   = w10 + s*w11 + s2*w12 + v2*w13                  (w1p = w_paths1[e,p,c])
    y0   = W_lin0^T out0 ;  y1_i = W_lin1^T (c1 * v_i)
    out[n] = [y0 | interleave_i(y1_i)]

v4 (from hardware profiling of v3):
  * Species-pure 1024-node chunks (13/core): halves instruction count
    and per-op fixed costs vs 512 (scalar affine -29%), halves the
    per-chunk stationary/vec DMA bytes. PE matmuls + PSUM copies still
    run on 512-wide halves (PSUM bank limit).
  * y0 path weights folded into per-chunk DMA'd stationaries
    W0p[c,m] = w0p[c]*W_lin0[c,m] -> y0 = 5 PE streams (s,s2,v2,s3,s*v2),
    no per-path elementwise multiplies, single fixed SPMD program.
  * y1 via c1 chain: scalar affine u2, tA; DVE STT tB; gpsimd add c1;
    DVE z products. Software-pipelined one chunk deep so the long
    c1->z->y1->copy chain never head-of-line blocks the next chunk.
  * Dtypes per-op: fp16 planes (precision), bf16 where DVE vec-scalar
    ops are involved (fp16 + f32 vec triggers a 2.3us CAST bug).
  * DMA rings: feats on sync, stats+output on gpsimd, nothing on the
    busy scalar queue.
"""

import sys

sys.path.insert(0, "/opt/trn_rl_repo")

from contextlib import ExitStack

import ml_dtypes
import numpy as np

import concourse.bass as bass
import concourse.tile as tile
from concourse import bacc, mybir
from concourse.bass_utils import run_bass_kernel_spmd

N_CORES = 8
N_NODES = 100000
C = 128
S = 10
CHUNK = 1024
HALF = 512
NCH = 13                      # chunks per core
PER_CORE = NCH * CHUNK        # 13312
TOTAL_CH = N_CORES * NCH      # 104 chunks of 1024
F32 = mybir.dt.float32
F16 = mybir.dt.float16
BF16 = mybir.dt.bfloat16


def build_bass():
    nc = bacc.Bacc()
    featsH = nc.dram_tensor("featsH", (4, C, PER_CORE), F16, kind="ExternalInput")
    featsV2 = nc.dram_tensor("featsV2", (C, PER_CORE), BF16, kind="ExternalInput")
    vecs = nc.dram_tensor("vecs", (C, NCH * 4), F32, kind="ExternalInput")
    w0s = nc.dram_tensor("w0s", (NCH, C, 1280), mybir.dt.uint8, kind="ExternalInput")
    w1h = nc.dram_tensor("w1h", (C, C), F16, kind="ExternalInput")
    out = nc.dram_tensor("out", (C, 4, PER_CORE), F16, kind="ExternalOutput")

    with tile.TileContext(nc) as tc, ExitStack() as ctx:
        _body(ctx, tc, featsH, featsV2, vecs, w0s, w1h, out)
    nc.finalize()
    return nc


def _body(ctx, tc, featsH, featsV2, vecs, w0s, w1h, out):
    nc = tc.nc
    mult = mybir.AluOpType.mult
    add = mybir.AluOpType.add

    const = ctx.enter_context(tc.tile_pool(name="const", bufs=1))
    io = ctx.enter_context(tc.tile_pool(name="io", bufs=3))
    st = ctx.enter_context(tc.tile_pool(name="st", bufs=3))
    ew = ctx.enter_context(tc.tile_pool(name="ew", bufs=3))
    ysb = ctx.enter_context(tc.tile_pool(name="ysb", bufs=2))
    ps = ctx.enter_context(tc.tile_pool(name="ps", bufs=2, space="PSUM"))

    vecs_sb = const.tile([C, NCH * 4], F32)
    nc.sync.dma_start(out=vecs_sb, in_=vecs[:, :])
    w1_sb = const.tile([C, C], F16)
    nc.sync.dma_start(out=w1_sb, in_=w1h[:, :])

    # engine warmups: absorb one-time TENSOR_LOAD / ACT table load while
    # the first feats DMA is in flight (no data dependencies)
    wa = const.tile([C, 64], F16)
    nc.vector.memset(wa, 1.0)
    wb = const.tile([C, 64], F16)
    nc.vector.tensor_tensor(out=wb, in0=wa, in1=wa, op=mult)
    wc = const.tile([C, 64], F16)
    nc.scalar.activation(
        wc, wa, mybir.ActivationFunctionType.Identity, bias=0.0, scale=1.0
    )
    wd = const.tile([C, 64], F16)
    nc.gpsimd.memset(wd, 0.0)

    ftiles = {}
    stiles = {}
    affines = {}
    ypair = [None]
    state = {}

    def pre_early(ci):
        # scalar affine ops for chunk ci: inputs landed a full iteration ago,
        # so issuing them ahead of the previous chunk's copies keeps the
        # scalar queue from stalling on the y1-matmul chain.
        fH, v2 = ftiles[ci]
        s = fH[:, 0, :]

        def vec(k):
            col = ci * 4 + k
            return vecs_sb[:, col : col + 1]

        u2 = ew.tile([C, CHUNK], F16, tag="u2")
        nc.scalar.mul(u2, s, vec(1))
        tA = ew.tile([C, CHUNK], BF16, tag="tA")
        nc.scalar.activation(
            tA, v2, mybir.ActivationFunctionType.Identity, bias=vec(0), scale=vec(3)
        )
        affines[ci] = (u2, tA)

    def fetch(ci):
        n0 = ci * CHUNK
        if ci == 0 or ci % 2 == 1:
            span = 1 if ci == 0 else min(2, NCH - ci)
            fH2 = io.tile([C, 4, 2 * CHUNK], F16, tag="fH")
            nc.sync.dma_start(
                out=fH2[:, :, : span * CHUNK],
                in_=featsH[:, :, n0 : n0 + span * CHUNK].rearrange("k c n -> c k n"),
            )
            fV2 = io.tile([C, 2 * CHUNK], BF16, tag="fV")
            nc.sync.dma_start(
                out=fV2[:, : span * CHUNK], in_=featsV2[:, n0 : n0 + span * CHUNK]
            )
            for t in range(span):
                ftiles[ci + t] = (
                    fH2[:, :, t * CHUNK : (t + 1) * CHUNK],
                    fV2[:, t * CHUNK : (t + 1) * CHUNK],
                )
        if ci % 2 == 0:
            span = min(2, NCH - ci)
            stS2 = st.tile([C, 2, 1280], mybir.dt.uint8, tag="stS")
            nc.gpsimd.dma_start(
                out=stS2[:, :span, :],
                in_=w0s[ci : ci + span].rearrange("t c m -> c t m"),
            )
            for t in range(span):
                row = stS2[:, t, :]
                stiles[ci + t] = (
                    row[:, 0:768].bitcast(F16),
                    row[:, 768:1280].bitcast(BF16),
                )

    def early(ci):
        stH, stB = stiles.pop(ci)
        fH, v2 = ftiles.pop(ci)
        s = fH[:, 0, :]

        def vec(k):
            col = ci * 4 + k
            return vecs_sb[:, col : col + 1]

        u2, tA = affines.pop(ci)

        # DVE: squares/cubics for y0 streams, c1 chain
        s2 = ew.tile([C, CHUNK], BF16, tag="s2")
        nc.vector.tensor_tensor(out=s2, in0=s, in1=s, op=mult)
        s3 = ew.tile([C, CHUNK], F16, tag="s3")
        nc.vector.tensor_tensor(out=s3, in0=s, in1=s2, op=mult)
        sv2 = ew.tile([C, CHUNK], F16, tag="sv2")
        nc.vector.tensor_tensor(out=sv2, in0=s, in1=v2, op=mult)
        tB = ew.tile([C, CHUNK], F16, tag="tB")
        nc.vector.scalar_tensor_tensor(tB, s2, vec(2), tA, mult, add)
        c1 = ew.tile([C, CHUNK], F16, tag="c1")
        nc.vector.tensor_tensor(out=c1, in0=u2, in1=tB, op=add)

        yps = []
        for h in range(2):
            hs = slice(h * HALF, (h + 1) * HALF)
            y_ps = ps.tile([C, 4, HALF], F32, tag="y")
            nc.tensor.matmul(y_ps[:, 0, :], lhsT=stH[:, 0:C], rhs=s[:, hs],
                             start=True, stop=False)
            nc.tensor.matmul(y_ps[:, 0, :], lhsT=stB[:, 0:C], rhs=s2[:, hs],
                             start=False, stop=False)
            nc.tensor.matmul(y_ps[:, 0, :], lhsT=stB[:, C : 2 * C], rhs=v2[:, hs],
                             start=False, stop=False)
            nc.tensor.matmul(y_ps[:, 0, :], lhsT=stH[:, C : 2 * C], rhs=s3[:, hs],
                             start=False, stop=False)
            nc.tensor.matmul(y_ps[:, 0, :], lhsT=stH[:, 2 * C : 3 * C], rhs=sv2[:, hs],
                             start=False, stop=True)
            yps.append(y_ps)
        state[ci] = (fH, c1, yps)

    def late(ci, final=False):
        n0 = ci * CHUNK
        fH, c1, yps = state.pop(ci)
        vx = fH[:, 1, :]
        vy = fH[:, 2, :]
        vz = fH[:, 3, :]

        zx = ew.tile([C, CHUNK], F16, tag="zx")
        nc.vector.tensor_tensor(out=zx, in0=c1, in1=vx, op=mult)
        zy = ew.tile([C, CHUNK], F16, tag="zy")
        nc.vector.tensor_tensor(out=zy, in0=c1, in1=vy, op=mult)
        zz = ew.tile([C, CHUNK], F16, tag="zz")
        nc.vector.tensor_tensor(out=zz, in0=c1, in1=vz, op=mult)

        if ci % 2 == 0:
            y2 = ysb.tile([C, 4, 2 * CHUNK], F16, tag="y2")
            ypair[0] = y2
        y2 = ypair[0]
        base = (ci % 2) * CHUNK
        for h in range(2):
            hs = slice(h * HALF, (h + 1) * HALF)
            ys = slice(base + h * HALF, base + (h + 1) * HALF)
            y_ps = yps[h]
            nc.tensor.matmul(y_ps[:, 1, :], lhsT=w1_sb, rhs=zx[:, hs],
                             start=True, stop=True)
            nc.tensor.matmul(y_ps[:, 2, :], lhsT=w1_sb, rhs=zy[:, hs],
                             start=True, stop=True)
            nc.tensor.matmul(y_ps[:, 3, :], lhsT=w1_sb, rhs=zz[:, hs],
                             start=True, stop=True)
            nc.scalar.copy(out=y2[:, :, ys], in_=y_ps)
            if final:
                nc.gpsimd.dma_start(
                    out=out[:, :, n0 + h * HALF : n0 + (h + 1) * HALF],
                    in_=y2[:, :, ys],
                )
        if not final and ci % 2 == 1:
            nc.gpsimd.dma_start(
                out=out[:, :, n0 - CHUNK : n0 + CHUNK], in_=y2
            )

    fetch(0)
    for ci in range(NCH):
        if ci + 1 < NCH:
            fetch(ci + 1)
        if ci >= 1:
            late(ci - 1)
        pre_early(ci)
        early(ci)
    late(NCH - 1, final=True)


_NC_CACHE = {}


def _get_nc():
    if "nc" not in _NC_CACHE:
        _NC_CACHE["nc"] = build_bass()
    return _NC_CACHE["nc"]


def _plan(node_attrs):
    """Species-pure chunk layout: gidx[slot] = node idx or -1, chunk species."""
    species = np.argmax(node_attrs, axis=1)
    order = np.argsort(species, kind="stable")
    sorted_species = species[order]
    gidx_parts = []
    chunk_species = []
    for sp in range(S):
        idx_s = order[sorted_species == sp]
        n_s = len(idx_s)
        nch = -(-n_s // CHUNK) if n_s else 0
        if nch:
            pad = nch * CHUNK - n_s
            gidx_parts.append(idx_s)
            gidx_parts.append(np.full(pad, -1, dtype=np.int64))
            chunk_species.extend([sp] * nch)
    n_used = len(chunk_species)
    assert n_used <= TOTAL_CH, n_used
    gidx_parts.append(np.full((TOTAL_CH - n_used) * CHUNK, -1, dtype=np.int64))
    chunk_species.extend([0] * (TOTAL_CH - n_used))
    gidx = np.concatenate(gidx_parts)
    return gidx, np.asarray(chunk_species)


def kernel(node_feats, node_attrs, w_paths0, w_paths1, W_lin0, W_lin1):
    n = node_feats.shape[0]
    assert n == N_NODES, n
    node_feats = np.asarray(node_feats, np.float32)
    gidx, chunk_species = _plan(np.asarray(node_attrs, np.float32))
    valid = gidx >= 0

    FS = np.zeros((TOTAL_CH * CHUNK, C, 4), dtype=np.float32)
    FS[valid] = node_feats[gidx[valid]]
    FT = FS.transpose(2, 1, 0)  # [4, C, slots] f32
    planesH = np.ascontiguousarray(FT).astype(np.float16)  # s, vx, vy, vz
    planeV2 = (FT[1] * FT[1] + FT[2] * FT[2] + FT[3] * FT[3]).astype(ml_dtypes.bfloat16)

    w0_tab = np.asarray(w_paths0, np.float32)  # [S, 5, C]
    w1_tab = np.asarray(w_paths1, np.float32)  # [S, 4, C]
    W0 = np.asarray(W_lin0, np.float32)
    # per-species folded stationaries: W0p[c,m] = w0p[c] * W0[c,m]
    spH = np.empty((S, C, 3 * C), dtype=np.float16)           # paths 0, 3, 4
    spB = np.empty((S, C, 2 * C), dtype=ml_dtypes.bfloat16)   # paths 1, 2
    for sp in range(S):
        for j, p in enumerate((0, 3, 4)):
            spH[sp, :, j * C : (j + 1) * C] = (w0_tab[sp, p][:, None] * W0).astype(
                np.float16
            )
        for j, p in enumerate((1, 2)):
            spB[sp, :, j * C : (j + 1) * C] = (w0_tab[sp, p][:, None] * W0).astype(
                ml_dtypes.bfloat16
            )
    w1h = np.ascontiguousarray(W_lin1, dtype=np.float16)
    # pack [3C fp16 | 2C bf16] stationary bytes per (species, c) row
    spPacked = np.empty((S, C, 1280), dtype=np.uint8)
    spPacked[:, :, 0:768] = spH.view(np.uint8).reshape(S, C, 768)
    spPacked[:, :, 768:1280] = spB.view(np.uint8).reshape(S, C, 512)

    nc = _get_nc()
    in_maps = []
    for k in range(N_CORES):
        cs = chunk_species[k * NCH : (k + 1) * NCH]
        sl = slice(k * PER_CORE, (k + 1) * PER_CORE)
        vt = np.empty((NCH, 4, C), dtype=np.float32)
        for ci, sp in enumerate(cs):
            vt[ci] = w1_tab[sp]  # w10, w11, w12, w13
        in_maps.append(
            {
                "featsH": np.ascontiguousarray(planesH[:, :, sl]),
                "featsV2": np.ascontiguousarray(planeV2[:, sl]),
                "vecs": np.ascontiguousarray(vt.reshape(NCH * 4, C).T),
                "w0s": np.ascontiguousarray(spPacked[cs]),
                "w1h": w1h,
            }
        )
    res = run_bass_kernel_spmd(nc, in_maps, core_ids=list(range(N_CORES)))
    full = np.concatenate(
        [res.results[k]["out"] for k in range(N_CORES)], axis=2
    )  # [C, 4, slots] fp16

    nidx = gidx[valid]
    outv = np.empty((N_NODES, 4 * C), dtype=np.float32)
    outv[nidx, 0:C] = full[:, 0, valid].T
    outv[nidx, C + 0 :: 3] = full[:, 1, valid].T
    outv[nidx, C + 1 :: 3] = full[:, 2, valid].T
    outv[nidx, C + 2 :: 3] = full[:, 3, valid].T
    return outv


# revision 27
# speedup vs baseline: 1.0535x; 1.0535x over previous
"""
Trainium2 Bass kernel for nn_EquivariantProductBasisBlock (v4).

Math (per node n, channel c), species e = argmax(attrs[n]):
    s = feats[n,c,0]; v = feats[n,c,1:4]; s2 = s^2; v2 = |v|^2
    out0 = s*w00 + s2*w01 + v2*w02 + s^3*w03 + s*v2*w04   (w0p = w_paths0[e,p,c])
    c1   = w10 + s*w11 + s2*w12 + v2*w13                  (w1p = w_paths1[e,p,c])
    y0   = W_lin0^T out0 ;  y1_i = W_lin1^T (c1 * v_i)
    out[n] = [y0 | interleave_i(y1_i)]

v4 (from hardware profiling of v3):
  * Species-pure 1024-node chunks (13/core): halves instruction count
    and per-op fixed costs vs 512 (scalar affine -29%), halves the
    per-chunk stationary/vec DMA bytes. PE matmuls + PSUM copies still
    run on 512-wide halves (PSUM bank limit).
  * y0 path weights folded into per-chunk DMA'd stationaries
    W0p[c,m] = w0p[c]*W_lin0[c,m] -> y0 = 5 PE streams (s,s2,v2,s3,s*v2),
    no per-path elementwise multiplies, single fixed SPMD program.
  * y1 via c1 chain: scalar affine u2, tA; DVE STT tB; gpsimd add c1;
    DVE z products. Software-pipelined one chunk deep so the long
    c1->z->y1->copy chain never head-of-line blocks the next chunk.
  * Dtypes per-op: fp16 planes (precision), bf16 where DVE vec-scalar
    ops are involved (fp16 + f32 vec triggers a 2.3us CAST bug).
  * DMA rings: feats on sync, stats+output on gpsimd, nothing on the
    busy scalar queue.
"""

import sys

sys.path.insert(0, "/opt/trn_rl_repo")

from contextlib import ExitStack

import ml_dtypes
import numpy as np

import concourse.bass as bass
import concourse.tile as tile
from concourse import bacc, mybir
from concourse.bass_utils import run_bass_kernel_spmd

N_CORES = 8
N_NODES = 100000
C = 128
S = 10
CHUNK = 1024
HALF = 512
NCH = 13                      # chunks per core
PER_CORE = NCH * CHUNK        # 13312
TOTAL_CH = N_CORES * NCH      # 104 chunks of 1024
F32 = mybir.dt.float32
F16 = mybir.dt.float16
BF16 = mybir.dt.bfloat16


def build_bass():
    nc = bacc.Bacc()
    featsH = nc.dram_tensor("featsH", (4, C, PER_CORE), F16, kind="ExternalInput")
    featsV2 = nc.dram_tensor("featsV2", (C, PER_CORE), BF16, kind="ExternalInput")
    vecs = nc.dram_tensor("vecs", (C, NCH * 4), F32, kind="ExternalInput")
    w0sh = nc.dram_tensor("w0sh", (NCH, C, 3 * C), F16, kind="ExternalInput")
    w0sb = nc.dram_tensor("w0sb", (NCH, C, 2 * C), BF16, kind="ExternalInput")
    w1h = nc.dram_tensor("w1h", (C, C), F16, kind="ExternalInput")
    out = nc.dram_tensor("out", (C, 4, PER_CORE), F16, kind="ExternalOutput")

    with tile.TileContext(nc) as tc, ExitStack() as ctx:
        _body(ctx, tc, featsH, featsV2, vecs, w0sh, w0sb, w1h, out)
    nc.finalize()
    return nc


def _body(ctx, tc, featsH, featsV2, vecs, w0sh, w0sb, w1h, out):
    nc = tc.nc
    mult = mybir.AluOpType.mult
    add = mybir.AluOpType.add

    const = ctx.enter_context(tc.tile_pool(name="const", bufs=1))
    io = ctx.enter_context(tc.tile_pool(name="io", bufs=3))
    st = ctx.enter_context(tc.tile_pool(name="st", bufs=3))
    ew = ctx.enter_context(tc.tile_pool(name="ew", bufs=3))
    ysb = ctx.enter_context(tc.tile_pool(name="ysb", bufs=2))
    ps = ctx.enter_context(tc.tile_pool(name="ps", bufs=2, space="PSUM"))

    vecs_sb = const.tile([C, NCH * 4], F32)
    nc.sync.dma_start(out=vecs_sb, in_=vecs[:, :])
    w1_sb = const.tile([C, C], F16)
    nc.sync.dma_start(out=w1_sb, in_=w1h[:, :])

    # engine warmups: absorb one-time TENSOR_LOAD / ACT table load while
    # the first feats DMA is in flight (no data dependencies)
    wa = const.tile([C, 64], F16)
    nc.vector.memset(wa, 1.0)
    wb = const.tile([C, 64], F16)
    nc.vector.tensor_tensor(out=wb, in0=wa, in1=wa, op=mult)
    wc = const.tile([C, 64], F16)
    nc.scalar.activation(
        wc, wa, mybir.ActivationFunctionType.Identity, bias=0.0, scale=1.0
    )
    wd = const.tile([C, 64], F16)
    nc.gpsimd.memset(wd, 0.0)

    ftiles = {}
    stiles = {}
    affines = {}
    ypair = [None]
    state = {}

    def pre_early(ci):
        # scalar affine ops for chunk ci: inputs landed a full iteration ago,
        # so issuing them ahead of the previous chunk's copies keeps the
        # scalar queue from stalling on the y1-matmul chain.
        fH, v2 = ftiles[ci]
        s = fH[:, 0, :]

        def vec(k):
            col = ci * 4 + k
            return vecs_sb[:, col : col + 1]

        u2 = ew.tile([C, CHUNK], F16, tag="u2")
        nc.scalar.mul(u2, s, vec(1))
        tA = ew.tile([C, CHUNK], BF16, tag="tA")
        nc.scalar.activation(
            tA, v2, mybir.ActivationFunctionType.Identity, bias=vec(0), scale=vec(3)
        )
        affines[ci] = (u2, tA)

    def fetch(ci):
        n0 = ci * CHUNK
        if ci == 0 or ci % 2 == 1:
            span = 1 if ci == 0 else min(2, NCH - ci)
            fH2 = io.tile([C, 4, 2 * CHUNK], F16, tag="fH")
            nc.sync.dma_start(
                out=fH2[:, :, : span * CHUNK],
                in_=featsH[:, :, n0 : n0 + span * CHUNK].rearrange("k c n -> c k n"),
            )
            fV2 = io.tile([C, 2 * CHUNK], BF16, tag="fV")
            nc.sync.dma_start(
                out=fV2[:, : span * CHUNK], in_=featsV2[:, n0 : n0 + span * CHUNK]
            )
            for t in range(span):
                ftiles[ci + t] = (
                    fH2[:, :, t * CHUNK : (t + 1) * CHUNK],
                    fV2[:, t * CHUNK : (t + 1) * CHUNK],
                )
        if ci % 2 == 0:
            span = min(2, NCH - ci)
            stH2 = st.tile([C, 2, 3 * C], F16, tag="stH")
            nc.gpsimd.dma_start(
                out=stH2[:, :span, :],
                in_=w0sh[ci : ci + span].rearrange("t c m -> c t m"),
            )
            stB2 = st.tile([C, 2, 2 * C], BF16, tag="stB")
            nc.gpsimd.dma_start(
                out=stB2[:, :span, :],
                in_=w0sb[ci : ci + span].rearrange("t c m -> c t m"),
            )
            for t in range(span):
                stiles[ci + t] = (stH2[:, t, :], stB2[:, t, :])

    def early(ci):
        stH, stB = stiles.pop(ci)
        fH, v2 = ftiles.pop(ci)
        s = fH[:, 0, :]

        def vec(k):
            col = ci * 4 + k
            return vecs_sb[:, col : col + 1]

        u2, tA = affines.pop(ci)

        # DVE: squares/cubics for y0 streams, c1 chain
        s2 = ew.tile([C, CHUNK], BF16, tag="s2")
        nc.vector.tensor_tensor(out=s2, in0=s, in1=s, op=mult)
        s3 = ew.tile([C, CHUNK], F16, tag="s3")
        nc.vector.tensor_tensor(out=s3, in0=s, in1=s2, op=mult)
        sv2 = ew.tile([C, CHUNK], F16, tag="sv2")
        nc.vector.tensor_tensor(out=sv2, in0=s, in1=v2, op=mult)
        tB = ew.tile([C, CHUNK], F16, tag="tB")
        nc.vector.scalar_tensor_tensor(tB, s2, vec(2), tA, mult, add)
        c1 = ew.tile([C, CHUNK], F16, tag="c1")
        nc.vector.tensor_tensor(out=c1, in0=u2, in1=tB, op=add)

        yps = []
        for h in range(2):
            hs = slice(h * HALF, (h + 1) * HALF)
            y_ps = ps.tile([C, 4, HALF], F32, tag="y")
            nc.tensor.matmul(y_ps[:, 0, :], lhsT=stH[:, 0:C], rhs=s[:, hs],
                             start=True, stop=False)
            nc.tensor.matmul(y_ps[:, 0, :], lhsT=stB[:, 0:C], rhs=s2[:, hs],
                             start=False, stop=False)
            nc.tensor.matmul(y_ps[:, 0, :], lhsT=stB[:, C : 2 * C], rhs=v2[:, hs],
                             start=False, stop=False)
            nc.tensor.matmul(y_ps[:, 0, :], lhsT=stH[:, C : 2 * C], rhs=s3[:, hs],
                             start=False, stop=False)
            nc.tensor.matmul(y_ps[:, 0, :], lhsT=stH[:, 2 * C : 3 * C], rhs=sv2[:, hs],
                             start=False, stop=True)
            yps.append(y_ps)
        state[ci] = (fH, c1, yps)

    def late(ci, final=False):
        n0 = ci * CHUNK
        fH, c1, yps = state.pop(ci)
        vx = fH[:, 1, :]
        vy = fH[:, 2, :]
        vz = fH[:, 3, :]

        zx = ew.tile([C, CHUNK], F16, tag="zx")
        nc.vector.tensor_tensor(out=zx, in0=c1, in1=vx, op=mult)
        zy = ew.tile([C, CHUNK], F16, tag="zy")
        nc.vector.tensor_tensor(out=zy, in0=c1, in1=vy, op=mult)
        zz = ew.tile([C, CHUNK], F16, tag="zz")
        nc.vector.tensor_tensor(out=zz, in0=c1, in1=vz, op=mult)

        if ci % 2 == 0:
            y2 = ysb.tile([C, 4, 2 * CHUNK], F16, tag="y2")
            ypair[0] = y2
        y2 = ypair[0]
        base = (ci % 2) * CHUNK
        for h in range(2):
            hs = slice(h * HALF, (h + 1) * HALF)
            ys = slice(base + h * HALF, base + (h + 1) * HALF)
            y_ps = yps[h]
            nc.tensor.matmul(y_ps[:, 1, :], lhsT=w1_sb, rhs=zx[:, hs],
                             start=True, stop=True)
            nc.tensor.matmul(y_ps[:, 2, :], lhsT=w1_sb, rhs=zy[:, hs],
                             start=True, stop=True)
            nc.tensor.matmul(y_ps[:, 3, :], lhsT=w1_sb, rhs=zz[:, hs],
                             start=True, stop=True)
            nc.scalar.copy(out=y2[:, :, ys], in_=y_ps)
            if final:
                nc.gpsimd.dma_start(
                    out=out[:, :, n0 + h * HALF : n0 + (h + 1) * HALF],
                    in_=y2[:, :, ys],
                )
        if not final and ci % 2 == 1:
            nc.gpsimd.dma_start(
                out=out[:, :, n0 - CHUNK : n0 + CHUNK], in_=y2
            )

    fetch(0)
    for ci in range(NCH):
        if ci + 1 < NCH:
            fetch(ci + 1)
        if ci >= 1:
            late(ci - 1)
        pre_early(ci)
        early(ci)
    late(NCH - 1, final=True)


_NC_CACHE = {}


def _get_nc():
    if "nc" not in _NC_CACHE:
        _NC_CACHE["nc"] = build_bass()
    return _NC_CACHE["nc"]


def _plan(node_attrs):
    """Species-pure chunk layout: gidx[slot] = node idx or -1, chunk species."""
    species = np.argmax(node_attrs, axis=1)
    order = np.argsort(species, kind="stable")
    sorted_species = species[order]
    gidx_parts = []
    chunk_species = []
    for sp in range(S):
        idx_s = order[sorted_species == sp]
        n_s = len(idx_s)
        nch = -(-n_s // CHUNK) if n_s else 0
        if nch:
            pad = nch * CHUNK - n_s
            gidx_parts.append(idx_s)
            gidx_parts.append(np.full(pad, -1, dtype=np.int64))
            chunk_species.extend([sp] * nch)
    n_used = len(chunk_species)
    assert n_used <= TOTAL_CH, n_used
    gidx_parts.append(np.full((TOTAL_CH - n_used) * CHUNK, -1, dtype=np.int64))
    chunk_species.extend([0] * (TOTAL_CH - n_used))
    gidx = np.concatenate(gidx_parts)
    return gidx, np.asarray(chunk_species)


def kernel(node_feats, node_attrs, w_paths0, w_paths1, W_lin0, W_lin1):
    n = node_feats.shape[0]
    assert n == N_NODES, n
    node_feats = np.asarray(node_feats, np.float32)
    gidx, chunk_species = _plan(np.asarray(node_attrs, np.float32))
    valid = gidx >= 0

    FS = np.zeros((TOTAL_CH * CHUNK, C, 4), dtype=np.float32)
    FS[valid] = node_feats[gidx[valid]]
    FT = FS.transpose(2, 1, 0)  # [4, C, slots] f32
    planesH = np.ascontiguousarray(FT).astype(np.float16)  # s, vx, vy, vz
    planeV2 = (FT[1] * FT[1] + FT[2] * FT[2] + FT[3] * FT[3]).astype(ml_dtypes.bfloat16)

    w0_tab = np.asarray(w_paths0, np.float32)  # [S, 5, C]
    w1_tab = np.asarray(w_paths1, np.float32)  # [S, 4, C]
    W0 = np.asarray(W_lin0, np.float32)
    # per-species folded stationaries: W0p[c,m] = w0p[c] * W0[c,m]
    spH = np.empty((S, C, 3 * C), dtype=np.float16)           # paths 0, 3, 4
    spB = np.empty((S, C, 2 * C), dtype=ml_dtypes.bfloat16)   # paths 1, 2
    for sp in range(S):
        for j, p in enumerate((0, 3, 4)):
            spH[sp, :, j * C : (j + 1) * C] = (w0_tab[sp, p][:, None] * W0).astype(
                np.float16
            )
        for j, p in enumerate((1, 2)):
            spB[sp, :, j * C : (j + 1) * C] = (w0_tab[sp, p][:, None] * W0).astype(
                ml_dtypes.bfloat16
            )
    w1h = np.ascontiguousarray(W_lin1, dtype=np.float16)

    nc = _get_nc()
    in_maps = []
    for k in range(N_CORES):
        cs = chunk_species[k * NCH : (k + 1) * NCH]
        sl = slice(k * PER_CORE, (k + 1) * PER_CORE)
        vt = np.empty((NCH, 4, C), dtype=np.float32)
        for ci, sp in enumerate(cs):
            vt[ci] = w1_tab[sp]  # w10, w11, w12, w13
        in_maps.append(
            {
                "featsH": np.ascontiguousarray(planesH[:, :, sl]),
                "featsV2": np.ascontiguousarray(planeV2[:, sl]),
                "vecs": np.ascontiguousarray(vt.reshape(NCH * 4, C).T),
                "w0sh": np.ascontiguousarray(spH[cs]),
                "w0sb": np.ascontiguousarray(spB[cs]),
                "w1h": w1h,
            }
        )
    res = run_bass_kernel_spmd(nc, in_maps, core_ids=list(range(N_CORES)))
    full = np.concatenate(
        [res.results[k]["out"] for k in range(N_CORES)], axis=2
    )  # [C, 4, slots] fp16

    nidx = gidx[valid]
    outv = np.empty((N_NODES, 4 * C), dtype=np.float32)
    outv[nidx, 0:C] = full[:, 0, valid].T
    outv[nidx, C + 0 :: 3] = full[:, 1, valid].T
    outv[nidx, C + 1 :: 3] = full[:, 2, valid].T
    outv[nidx, C + 2 :: 3] = full[:, 3, valid].T
    return outv
